# revision 4
# baseline (speedup 1.0000x reference)
"""Multi-head self-attention (B=2, C=512, H=W=64, 8 heads) on 8 TRN2 cores.

Sharding: core i handles batch b = i//4 and pixel quarter q = i%4 (1024 of
4096 pixels) for ALL 8 heads.  K/V projections cover the full pixel range
(attention context), Q only the local quarter.  No collectives.

fp8 DoubleRow design (vs the bf16 baseline): all projections and the AV
matmul run in fp8 with perf_mode=DoubleRow (0.5 cycles/row, 2x contraction
per instruction -> 4x cheaper than bf16 on contraction-split matmuls).

  x      host-cast to e4m3 and c-folded: xf[p, cb*4096+n] = x[cb*128+p, n]
         so a DR matmul contracts c-pairs (c = p + 128i + 256cp) via 3-D
         APs [[pitch,128],[4096,2],[1,n]].  Weights are host-scaled x16
         (lifts them out of e4m3's subnormal range) and host-folded.
  K,Q:   DR matmuls, out [d(128), m]; evac: tensor_scalar x(1/16) (+bq for
         Q) -> bf16.  K bias dropped (softmax-invariant).
  V:     DR, out [m(128), d-all-heads]; evac ACT Copy x(1/16) -> e4m3 into
         pair tiles vsb[t][p, i*512 + h*64 + d] (i = chunk parity).
  S^T:   bf16 as in the baseline (2 heads row-tiled per group); per-head
         processing, chunks of 128 keys -> PSUM [128, 1024] f32.
  exp:   P = exp(s/8 - ln8) -> e5m2 (shift keeps P <= 137 inside e5m2/e4m3
         range; any fixed shift cancels in softmax).  Split ACT (true Exp,
         scale+bias, e5m2 out) / DVE (Schraudolph: bits = round(A*s + B)
         as int8, bitcast e5m2; HW-verified bit-exact vs numpy).  Both
         write halves of a pair tile pexp[p, i*1024 + n].
  AV:    DR e5m2 x e4m3: stationary P [[pitch,128],[1024,2],[1,128]],
         moving V [[pitch,128],[512,2],[1,64]], accumulating [n,d] over 16
         m-pair chunks; Z via DR ones-moving [[pitch,128],[1,2],[1,1]].
  norm:  rz = reciprocal(Z); per-nb scale into asb (bf16); v-bias deferred
         to the transpose evac (per-partition there).
  out:   PE-transpose asb -> [c, n] bf16 PSUM; evac +bv -> aoT e4m3 with
         c-pairs in column halves (aoT[cp][p, i*1024+n]); out-proj DR;
         epilogue (psum * gamma/16 + xr) -> DMA out.

PSUM (8 banks): S 2x[128,1024]f32 = 4, AV 2x1 (reused for the transpose),
Z 1, proj transients 1.
"""

import numpy as np
import ml_dtypes

import concourse.bass as bass
import concourse.mybir as mybir
import concourse.tile as tile
from concourse import bacc
from concourse import masks
from concourse.bass_utils import run_bass_kernel_spmd

F32 = mybir.dt.float32
BF16 = mybir.dt.bfloat16
E4 = mybir.dt.float8e4
E5 = mybir.dt.float8e5
I8 = mybir.dt.int8
AF = mybir.ActivationFunctionType
ALU = mybir.AluOpType
DR = mybir.MatmulPerfMode.DoubleRow

E4n = ml_dtypes.float8_e4m3
E5n = ml_dtypes.float8_e5m2

B, C, H, W = 2, 512, 64, 64
N = H * W              # 4096 pixels
NH, HD = 8, 64         # heads, head dim
NSL = N // 4           # pixels per core (queries)
NG = NH // 2           # head groups of 2
MT = N // 128          # m-chunks (key pixels)
NB = NSL // 128        # n-blocks (AV output partitions)
SW = 16.0              # host weight scale (e4m3 subnormal dodge)
SHIFT = float(np.log(8.0))          # P = exp(s/8 - ln8): e5m2-safe
EXP_A = 4.0 * float(np.log2(np.e)) / 8.0   # Schraudolph (1/8 folded)
EXP_B = 60.0 - 12.0 + 0.44                  # e5m2 bias - 4*shift + rounding

# exp engine split: ACT gets ACT_NUM of every ACT_DEN chunks (rest DVE)
ACT_NUM, ACT_DEN = 142, 256

_cached = {}


def _ap3(t, offset, step2, inner):
    """3-D folded view [[pitch,128],[step2,2],[1,inner]] of tile t."""
    a = t[:]
    return bass.AP(tensor=a.tensor, offset=a.offset + offset,
                   ap=[[a.ap[0][0], 128], [step2, 2], [1, inner]])


def _build_kernel():
    nc = bacc.Bacc("TRN2", target_bir_lowering=False, debug=False,
                   num_devices=8)

    xf = nc.dram_tensor("xf", [128, 4 * N], E4, kind="ExternalInput")
    xr = nc.dram_tensor("xr", [C, NSL], F32, kind="ExternalInput")
    wqkf = nc.dram_tensor("wqkf", [128, 4096], E4, kind="ExternalInput")
    wvf = nc.dram_tensor("wvf", [128, 2048], E4, kind="ExternalInput")
    wof = nc.dram_tensor("wof", [128, 2048], E4, kind="ExternalInput")
    bq = nc.dram_tensor("bq", [C, 1], F32, kind="ExternalInput")
    bvc = nc.dram_tensor("bvc", [C, 1], F32, kind="ExternalInput")
    gam = nc.dram_tensor("gam", [1, 1], F32, kind="ExternalInput")
    out = nc.dram_tensor("out", [C, NSL], F32, kind="ExternalOutput")

    with tile.TileContext(nc) as tc:
        _emit_body(nc, tc, xf, xr, wqkf, wvf, wof, bq, bvc, gam, out)
    nc.compile()
    return nc


def _emit_body(nc, tc, xf, xr, wqkf, wvf, wof, bq, bvc, gam, out):
    from contextlib import ExitStack
    with ExitStack() as ctx:
        ep = ctx.enter_context

        consts = ep(tc.tile_pool(name="consts", bufs=1))
        xrp = ep(tc.tile_pool(name="xrp", bufs=1))
        kq = ep(tc.tile_pool(name="kq", bufs=1))
        vp = ep(tc.tile_pool(name="vp", bufs=1))
        pexp = ep(tc.tile_pool(name="pexp", bufs=6))
        asbp = ep(tc.tile_pool(name="asbp", bufs=2))
        rzp = ep(tc.tile_pool(name="rzp", bufs=4))
        aop = ep(tc.tile_pool(name="aop", bufs=1))
        epi = ep(tc.tile_pool(name="epi", bufs=4))
        ps_s = ep(tc.tile_pool(name="ps_s", bufs=2, space="PSUM"))
        ps_av = ep(tc.tile_pool(name="ps_av", bufs=2, space="PSUM"))
        ps_z = ep(tc.tile_pool(name="ps_z", bufs=1, space="PSUM"))
        ps_m = ep(tc.tile_pool(name="ps_m", bufs=1, space="PSUM"))

        # ---- constants + input loads -------------------------------------
        wqkt = consts.tile([128, 4096], E4, tag="wqkt")
        nc.sync.dma_start(out=wqkt, in_=wqkf[:, :])
        bqs = []
        bvs = []
        for g in range(NG):
            t = consts.tile([128, 1], F32, tag=f"bq{g}", name=f"bqt{g}")
            nc.gpsimd.dma_start(out=t, in_=bq[g * 128:(g + 1) * 128, :])
            bqs.append(t)
            t = consts.tile([128, 1], F32, tag=f"bv{g}", name=f"bvt{g}")
            nc.gpsimd.dma_start(out=t, in_=bvc[g * 128:(g + 1) * 128, :])
            bvs.append(t)
        xft = consts.tile([128, 4 * N], E4, tag="xft")
        # queries (m-slices 0..1) first, then the key tail
        for ms in range(8):
            for cb in range(4):
                o = cb * N + ms * 512
                nc.sync.dma_start(out=xft[:, o:o + 512], in_=xf[:, o:o + 512])
        wvt = consts.tile([128, 2048], E4, tag="wvt")
        nc.sync.dma_start(out=wvt, in_=wvf[:, :])
        wot = consts.tile([128, 2048], E4, tag="wot")
        nc.sync.dma_start(out=wot, in_=wof[:, :])
        gam128 = consts.tile([128, 1], F32, tag="gam")
        ga = gam.ap()
        nc.gpsimd.dma_start(
            out=gam128,
            in_=bass.AP(tensor=ga.tensor, offset=ga.offset,
                        ap=[[0, 128], [1, 1]]))
        ebias = consts.tile([128, 1], F32, tag="ebias")
        nc.vector.memset(ebias, -SHIFT)
        ones2 = consts.tile([128, 2], E4, tag="ones2")
        nc.vector.memset(ones2, 1.0)
        ident = consts.tile([128, 128], BF16, tag="ident")
        masks.make_identity(nc, ident[:])
        xrs = []
        for cb in range(4):
            t = xrp.tile([128, NSL], F32, tag=f"xr{cb}", name=f"xrt{cb}")
            nc.sync.dma_start(out=t, in_=xr[cb * 128:(cb + 1) * 128, :])
            xrs.append(t)

        # ---- persistent SBUF state --------------------------------------
        ksb = [kq.tile([128, N], BF16, tag=f"k{g}", name=f"k{g}")
               for g in range(NG)]
        qsb = [kq.tile([128, NSL], BF16, tag=f"q{g}", name=f"q{g}")
               for g in range(NG)]
        vsb = [vp.tile([128, 1024], E4, tag=f"v{t}", name=f"v{t}")
               for t in range(MT // 2)]
        aoT = [aop.tile([128, 2048], E4, tag=f"aoT{cp}", name=f"aoT{cp}")
               for cp in range(2)]

        def kq_piece(g, piece, qk):
            # Q (qk=0, pieces 0..1, n-cols) / K (qk=1, pieces 0..7, m-cols)
            p = ps_m.tile([128, 512], F32, tag="ps_m",
                          name=f"kq{qk}_{g}_{piece}")
            for cp in range(2):
                stat = _ap3(wqkt, ((g * 2 + qk) * 2 + cp) * 256, 128, 128)
                for hf in range(2):
                    mov = _ap3(xft, (2 * cp) * N + piece * 512 + hf * 256,
                               N, 256)
                    nc.tensor.matmul(
                        p[:, hf * 256:(hf + 1) * 256], stat, mov,
                        start=(cp == 0 and hf == 0), stop=(cp == 1),
                        perf_mode=DR, skip_group_check=True)
            sl = slice(piece * 512, (piece + 1) * 512)
            if qk == 0:
                nc.vector.tensor_scalar(qsb[g][:, sl], p, 1.0 / SW, bqs[g],
                                        ALU.mult, ALU.add)
            else:
                nc.vector.tensor_scalar_mul(ksb[g][:, sl], p, 1.0 / SW)

        def v_piece(c):
            # m-chunk c -> vsb[c//2] half (c%2): all 8 heads' V, e4m3
            p = ps_m.tile([128, 512], F32, tag="ps_m", name=f"v{c}")
            for cp in range(2):
                stat = _ap3(xft, (2 * cp) * N + c * 128, N, 128)
                for dw in range(2):
                    mov = _ap3(wvt, cp * 1024 + dw * 256, 512, 256)
                    nc.tensor.matmul(
                        p[:, dw * 256:(dw + 1) * 256], stat, mov,
                        start=(cp == 0 and dw == 0), stop=(cp == 1),
                        perf_mode=DR, skip_group_check=True)
            half = (c % 2) * 512
            nc.scalar.activation(vsb[c // 2][:, half:half + 512], p,
                                 AF.Copy, scale=1.0 / SW)

        def s_chunk(h, mt, pe):
            # S^T chunk [128 m, 1024 n] for head h + exp into pair tile half
            g, lo = h // 2, (h % 2) * 64
            sp = ps_s.tile([128, 1024], F32, tag="ps_s",
                           name=f"s{h}_{mt}")
            for ntt in range(2):
                nc.tensor.matmul(
                    sp[:, ntt * 512:(ntt + 1) * 512],
                    ksb[g][lo:lo + 64, mt * 128:(mt + 1) * 128],
                    qsb[g][lo:lo + 64, ntt * 512:(ntt + 1) * 512],
                    start=True, stop=True, tile_position=(lo, 0))
            half = (mt % 2) * 1024
            idx = h * MT + mt
            is_act = ((idx + 1) * ACT_NUM) // ACT_DEN \
                != (idx * ACT_NUM) // ACT_DEN
            if is_act:
                nc.scalar.activation(pe[:, half:half + 1024], sp, AF.Exp,
                                     scale=0.125, bias=ebias[:])
            else:
                nc.vector.tensor_scalar(
                    pe[:, half:half + 1024].bitcast(I8), sp,
                    EXP_A, EXP_B, ALU.mult, ALU.add)

        def av_pair(h, t, pe, avp, zp):
            for nb in range(NB):
                stat = _ap3(pe, nb * 128, 1024, 128)
                nc.tensor.matmul(
                    avp[:, nb * 64:(nb + 1) * 64], stat,
                    _ap3(vsb[t], h * 64, 512, 64),
                    start=(t == 0 and nb == 0), stop=(t == MT // 2 - 1),
                    perf_mode=DR, skip_group_check=True)
                nc.tensor.matmul(
                    zp[:, nb:nb + 1], stat,
                    bass.AP(tensor=ones2[:].tensor, offset=ones2[:].offset,
                            ap=[[ones2[:].ap[0][0], 128], [1, 2], [1, 1]]),
                    start=(t == 0 and nb == 0), stop=(t == MT // 2 - 1),
                    perf_mode=DR, skip_group_check=True)

        def norm(h, avp, zp, asb):
            lo = (h % 2) * 64
            rz = rzp.tile([128, NB], F32, tag="rz", name=f"rz{h}")
            nc.vector.reciprocal(rz, zp)
            for nb in range(NB):
                o = nb * 128 + lo
                if h % 2 == 0:
                    nc.scalar.activation(asb[:, o:o + 64],
                                         avp[:, nb * 64:(nb + 1) * 64],
                                         AF.Copy, scale=rz[:, nb:nb + 1])
                else:
                    nc.vector.tensor_scalar_mul(
                        asb[:, o:o + 64], avp[:, nb * 64:(nb + 1) * 64],
                        rz[:, nb:nb + 1])

        def group_end(g, asb):
            tp = ps_av.tile([128, NSL], BF16, tag="ps_av", name=f"tp{g}")
            for nb in range(NB):
                nc.tensor.matmul(
                    tp[:, nb * 128:(nb + 1) * 128],
                    asb[:, nb * 128:(nb + 1) * 128], ident,
                    is_transpose=True, start=(nb == 0), stop=(nb == NB - 1),
                    skip_group_check=True)
            half = (g % 2) * 1024
            nc.scalar.activation(aoT[g // 2][:, half:half + 1024], tp,
                                 AF.Identity, bias=bvs[g], scale=1.0)

        def out_proj(ntt, ot):
            po = ps_m.tile([128, 512], F32, tag="ps_m",
                           name=f"op{ot}_{ntt}")
            for cp in range(2):
                stat = _ap3(wot, (cp * 4 + ot) * 256, 128, 128)
                for hf in range(2):
                    mov = _ap3(aoT[cp], ntt * 512 + hf * 256, 1024, 256)
                    nc.tensor.matmul(
                        po[:, hf * 256:(hf + 1) * 256], stat, mov,
                        start=(cp == 0 and hf == 0), stop=(cp == 1),
                        perf_mode=DR, skip_group_check=True)
            t2 = epi.tile([128, 512], F32, tag="t2")
            nc.vector.scalar_tensor_tensor(
                out=t2, in0=po, scalar=gam128,
                in1=xrs[ot][:, ntt * 512:(ntt + 1) * 512],
                op0=ALU.mult, op1=ALU.add)
            nc.sync.dma_start(
                out=out[ot * 128:(ot + 1) * 128,
                        ntt * 512:(ntt + 1) * 512],
                in_=t2)

        # ---- preamble ----------------------------------------------------
        kq_piece(0, 0, 0)
        kq_piece(0, 1, 0)
        kq_piece(0, 0, 1)
        v_piece(0)
        v_piece(1)

        # ---- attention ---------------------------------------------------
        asb = None
        for h in range(NH):
            g = h // 2
            avp = ps_av.tile([128, NB * 64], F32, tag="ps_av",
                             name=f"av{h}")
            zp = ps_z.tile([128, NB], F32, tag="ps_z", name=f"z{h}")
            if h % 2 == 0:
                asb = asbp.tile([128, NSL], BF16, tag="asb",
                                name=f"asb{g}")
            pes = {}
            pe = None
            for mt in range(MT):
                if mt % 2 == 0:
                    pe = pexp.tile([128, 2048], E5, tag="pe",
                                   name=f"pe{h}_{mt}")
                    pes[mt // 2] = pe
                s_chunk(h, mt, pe)
                # interleaved projections
                if h == 0:
                    if 1 <= mt <= 7:
                        kq_piece(0, mt, 1)          # K(0) pieces 1..7
                    if mt <= 29:
                        v_piece(mt + 2)             # V pieces 2..31
                if h % 2 == 1 and h < NH - 1:
                    gn = g + 1
                    if mt < 2:
                        kq_piece(gn, mt, 0)         # Q(g+1)
                    elif mt < 10:
                        kq_piece(gn, mt - 2, 1)     # K(g+1) pieces 0..7
                # AV one pair behind the S fill
                if mt >= 3 and mt % 2 == 1:
                    av_pair(h, (mt - 3) // 2, pes[(mt - 3) // 2], avp, zp)
            av_pair(h, MT // 2 - 2, pes[MT // 2 - 2], avp, zp)
            av_pair(h, MT // 2 - 1, pes[MT // 2 - 1], avp, zp)
            norm(h, avp, zp, asb)
            if h % 2 == 1:
                group_end(g, asb)

        # ---- output projection ------------------------------------------
        for ntt in range(2):
            for ot in range(4):
                out_proj(ntt, ot)


def _prep_in_maps(x, w_qkv, b_qkv, w_out, b_out, gamma):
    x = np.asarray(x, np.float32).reshape(B, C, N)
    w_qkv = np.asarray(w_qkv, np.float32)
    b_qkv = np.asarray(b_qkv, np.float32)
    w_out = np.asarray(w_out, np.float32)
    b_out = np.asarray(b_out, np.float32)
    gamma = np.asarray(gamma, np.float32)

    # wqkf[p, ((g*2+qk)*2+cp)*256 + i*128 + d] = w_qkv[qk*C+g*128+d, c]*SW
    # with c = p + 128*i + 256*cp
    t = (w_qkv[:2 * C] * SW).reshape(2, 4, 128, 2, 2, 128)  # qk g d cp i p
    wqkf = np.ascontiguousarray(
        t.transpose(5, 1, 0, 3, 4, 2).reshape(128, 4096)).astype(E4n)
    # wvf[p, cp*1024 + i*512 + dall] = w_qkv[2C+dall, c]*SW
    t = (w_qkv[2 * C:] * SW).reshape(512, 2, 2, 128)        # dall cp i p
    wvf = np.ascontiguousarray(
        t.transpose(3, 1, 2, 0).reshape(128, 2048)).astype(E4n)
    # wof[p, (cp*4+ot)*256 + i*128 + o] = w_out[ot*128+o, c]*SW
    t = (w_out * SW).reshape(4, 128, 2, 2, 128)             # ot o cp i p
    wof = np.ascontiguousarray(
        t.transpose(4, 2, 0, 3, 1).reshape(128, 2048)).astype(E4n)

    bqcol = b_qkv[:C].reshape(C, 1)
    bvcol = b_qkv[2 * C:].reshape(C, 1)
    gam = (gamma / SW).reshape(1, 1)

    in_maps = []
    for i in range(8):
        b, q = i // 4, i % 4
        rot = np.roll(x[b], -q * NSL, axis=1)
        xf8 = np.ascontiguousarray(
            rot.reshape(4, 128, N).transpose(1, 0, 2).reshape(128, 4 * N)
        ).astype(E4n)
        xrs = np.ascontiguousarray(x[b][:, q * NSL:(q + 1) * NSL]) \
            + (gamma.reshape(()) * b_out)[:, None].astype(np.float32)
        in_maps.append({
            "xf": xf8, "xr": xrs, "wqkf": wqkf, "wvf": wvf, "wof": wof,
            "bq": bqcol, "bvc": bvcol, "gam": gam,
        })
    return in_maps


def _assemble(results):
    full = np.empty((B, C, N), np.float32)
    for i in range(8):
        b, q = i // 4, i % 4
        full[b][:, q * NSL:(q + 1) * NSL] = results[i]["out"]
    return full.reshape(B, C, H, W)


def kernel(x, w_qkv, b_qkv, w_out, b_out, gamma):
    if "nc" not in _cached:
        _cached["nc"] = _build_kernel()
    nc = _cached["nc"]
    in_maps = _prep_in_maps(x, w_qkv, b_qkv, w_out, b_out, gamma)
    res = run_bass_kernel_spmd(nc, in_maps, core_ids=list(range(8)))
    return _assemble(res.results)


# revision 10
# speedup vs baseline: 1.1690x; 1.1690x over previous
"""Multi-head self-attention (B=2, C=512, H=W=64, 8 heads) on 8 TRN2 cores.

Sharding: core i handles batch b = i//4 and pixel quarter q = i%4 (1024 of
4096 pixels) for ALL 8 heads.  K/V projections cover the full pixel range
(attention context), Q only the local quarter.  No collectives.

fp8 DoubleRow design (vs the bf16 baseline): all projections and the AV
matmul run in fp8 with perf_mode=DoubleRow (0.5 cycles/row, 2x contraction
per instruction -> 4x cheaper than bf16 on contraction-split matmuls).

  x      host-cast to e4m3 and c-folded: xf[p, cb*4096+n] = x[cb*128+p, n]
         so a DR matmul contracts c-pairs (c = p + 128i + 256cp) via 3-D
         APs [[pitch,128],[4096,2],[1,n]].  Weights are host-scaled x16
         (lifts them out of e4m3's subnormal range) and host-folded.
  K,Q:   DR matmuls, out [d(128), m]; evac: tensor_scalar x(1/16) (+bq for
         Q) -> bf16.  K bias dropped (softmax-invariant).
  V:     DR, out [m(128), d-all-heads]; evac ACT Copy x(1/16) -> e4m3 into
         pair tiles vsb[t][p, i*512 + h*64 + d] (i = chunk parity).
  S^T:   bf16 as in the baseline (2 heads row-tiled per group); per-head
         processing, chunks of 128 keys -> PSUM [128, 1024] f32.
  exp:   P = exp(s/8 - ln8) -> e5m2 (shift keeps P <= 137 inside e5m2/e4m3
         range; any fixed shift cancels in softmax).  Split ACT (true Exp,
         scale+bias, e5m2 out) / DVE (Schraudolph: bits = round(A*s + B)
         as int8, bitcast e5m2; HW-verified bit-exact vs numpy).  Both
         write halves of a pair tile pexp[p, i*1024 + n].
  AV:    DR e5m2 x e4m3: stationary P [[pitch,128],[1024,2],[1,128]],
         moving V [[pitch,128],[512,2],[1,64]], accumulating [n,d] over 16
         m-pair chunks; Z via DR ones-moving [[pitch,128],[1,2],[1,1]].
  norm:  rz = reciprocal(Z); per-nb scale into asb (bf16); v-bias deferred
         to the transpose evac (per-partition there).
  out:   PE-transpose asb -> [c, n] bf16 PSUM; evac +bv -> aoT e4m3 with
         c-pairs in column halves (aoT[cp][p, i*1024+n]); out-proj DR;
         epilogue (psum * gamma/16 + xr) -> DMA out.

PSUM (8 banks): S 2x[128,1024]f32 = 4, AV 2x1 (reused for the transpose),
Z 1, proj transients 1.
"""

import numpy as np
import ml_dtypes

import concourse.bass as bass
import concourse.mybir as mybir
import concourse.tile as tile
from concourse import bacc
from concourse import masks
from concourse.bass_utils import run_bass_kernel_spmd

F32 = mybir.dt.float32
BF16 = mybir.dt.bfloat16
E4 = mybir.dt.float8e4
E5 = mybir.dt.float8e5
I8 = mybir.dt.int8
AF = mybir.ActivationFunctionType
ALU = mybir.AluOpType
DR = mybir.MatmulPerfMode.DoubleRow

E4n = ml_dtypes.float8_e4m3
E5n = ml_dtypes.float8_e5m2

B, C, H, W = 2, 512, 64, 64
N = H * W              # 4096 pixels
NH, HD = 8, 64         # heads, head dim
NSL = N // 4           # pixels per core (queries)
NG = NH // 2           # head groups of 2
MT = N // 128          # m-chunks (key pixels)
NB = NSL // 128        # n-blocks (AV output partitions)
SW = 16.0              # host weight scale (e4m3 subnormal dodge)
SHIFT = float(np.log(8.0))          # P = exp(s/8 - ln8): e5m2-safe
EXP_A = 4.0 * float(np.log2(np.e)) / 8.0   # Schraudolph (1/8 folded)
EXP_B = 60.0 - 12.0 + 0.44                  # e5m2 bias - 4*shift + rounding

# exp engine split: ACT gets ACT_NUM of every ACT_DEN chunks (rest DVE)
ACT_NUM, ACT_DEN = 140, 256

_cached = {}


def _ap3(t, offset, step2, inner):
    """3-D folded view [[pitch,128],[step2,2],[1,inner]] of tile t."""
    a = t[:]
    return bass.AP(tensor=a.tensor, offset=a.offset + offset,
                   ap=[[a.ap[0][0], 128], [step2, 2], [1, inner]])


def _build_kernel():
    nc = bacc.Bacc("TRN2", target_bir_lowering=False, debug=False,
                   num_devices=8)

    xf = nc.dram_tensor("xf", [128, 4 * N], E4, kind="ExternalInput")
    xr = nc.dram_tensor("xr", [C, NSL], F32, kind="ExternalInput")
    wqkf = nc.dram_tensor("wqkf", [128, 4096], E4, kind="ExternalInput")
    wvf = nc.dram_tensor("wvf", [128, 2048], E4, kind="ExternalInput")
    wof = nc.dram_tensor("wof", [128, 2048], E4, kind="ExternalInput")
    bq = nc.dram_tensor("bq", [C, 1], F32, kind="ExternalInput")
    bvc = nc.dram_tensor("bvc", [C, 1], F32, kind="ExternalInput")
    gam = nc.dram_tensor("gam", [1, 1], F32, kind="ExternalInput")
    out = nc.dram_tensor("out", [C, NSL], F32, kind="ExternalOutput")

    with tile.TileContext(nc) as tc:
        _emit_body(nc, tc, xf, xr, wqkf, wvf, wof, bq, bvc, gam, out)
    nc.compile()
    return nc


def _emit_body(nc, tc, xf, xr, wqkf, wvf, wof, bq, bvc, gam, out):
    from contextlib import ExitStack
    with ExitStack() as ctx:
        ep = ctx.enter_context

        consts = ep(tc.tile_pool(name="consts", bufs=1))
        xrp = ep(tc.tile_pool(name="xrp", bufs=1))
        kq = ep(tc.tile_pool(name="kq", bufs=1))
        vp = ep(tc.tile_pool(name="vp", bufs=1))
        pexp = ep(tc.tile_pool(name="pexp", bufs=6))
        asbp = ep(tc.tile_pool(name="asbp", bufs=2))
        rzp = ep(tc.tile_pool(name="rzp", bufs=4))
        aop = ep(tc.tile_pool(name="aop", bufs=1))
        epi = ep(tc.tile_pool(name="epi", bufs=4))
        # ONE shared 3-slot rotation ([128,1024] f32 = 2 banks each) for S
        # chunks AND all projection transients: breaks the 2-slot WAR chain
        # (fill(mt+k) waits exp(mt)) that serialized fill+exp latencies.
        ps_s = ep(tc.tile_pool(name="ps_s", bufs=3, space="PSUM"))
        ps_av = ep(tc.tile_pool(name="ps_av", bufs=1, space="PSUM"))
        ps_z = ep(tc.tile_pool(name="ps_z", bufs=1, space="PSUM"))

        # ---- constants + input loads -------------------------------------
        wqkt = consts.tile([128, 4096], E4, tag="wqkt")
        nc.sync.dma_start(out=wqkt, in_=wqkf[:, :])
        bqs = []
        bvs = []
        for g in range(NG):
            t = consts.tile([128, 1], F32, tag=f"bq{g}", name=f"bqt{g}")
            nc.gpsimd.dma_start(out=t, in_=bq[g * 128:(g + 1) * 128, :])
            bqs.append(t)
            t = consts.tile([128, 1], F32, tag=f"bv{g}", name=f"bvt{g}")
            nc.gpsimd.dma_start(out=t, in_=bvc[g * 128:(g + 1) * 128, :])
            bvs.append(t)
        xft = consts.tile([128, 4 * N], E4, tag="xft")
        # queries (m-slices 0..1) first, then the key tail
        for ms in range(8):
            for cb in range(4):
                o = cb * N + ms * 512
                nc.sync.dma_start(out=xft[:, o:o + 512], in_=xf[:, o:o + 512])
        wvt = consts.tile([128, 2048], E4, tag="wvt")
        nc.sync.dma_start(out=wvt, in_=wvf[:, :])
        wot = consts.tile([128, 2048], E4, tag="wot")
        nc.sync.dma_start(out=wot, in_=wof[:, :])
        gam128 = consts.tile([128, 1], F32, tag="gam")
        ga = gam.ap()
        nc.gpsimd.dma_start(
            out=gam128,
            in_=bass.AP(tensor=ga.tensor, offset=ga.offset,
                        ap=[[0, 128], [1, 1]]))
        ebias = consts.tile([128, 1], F32, tag="ebias")
        nc.vector.memset(ebias, -SHIFT)
        ones2 = consts.tile([128, 2], E4, tag="ones2")
        nc.vector.memset(ones2, 1.0)
        ident = consts.tile([128, 128], BF16, tag="ident")
        masks.make_identity(nc, ident[:])
        xrs = []
        for cb in range(4):
            t = xrp.tile([128, NSL], F32, tag=f"xr{cb}", name=f"xrt{cb}")
            nc.sync.dma_start(out=t, in_=xr[cb * 128:(cb + 1) * 128, :])
            xrs.append(t)

        # ---- persistent SBUF state --------------------------------------
        ksb = [kq.tile([128, N], BF16, tag=f"k{g}", name=f"k{g}")
               for g in range(NG)]
        qsb = [kq.tile([128, NSL], BF16, tag=f"q{g}", name=f"q{g}")
               for g in range(NG)]
        vsb = [vp.tile([128, 1024], E4, tag=f"v{t}", name=f"v{t}")
               for t in range(MT // 2)]
        aoT = [aop.tile([128, 2048], E4, tag=f"aoT{cp}", name=f"aoT{cp}")
               for cp in range(2)]

        def kq_piece(g, piece, qk):
            # Q (qk=0, one 1024-n piece) / K (qk=1, pieces 0..3 of 1024 m)
            p = ps_s.tile([128, 1024], F32, tag="ps_s",
                          name=f"kq{qk}_{g}_{piece}")
            for cp in range(2):
                stat = _ap3(wqkt, ((g * 2 + qk) * 2 + cp) * 256, 128, 128)
                for hf in range(4):
                    mov = _ap3(xft, (2 * cp) * N + piece * 1024 + hf * 256,
                               N, 256)
                    nc.tensor.matmul(
                        p[:, hf * 256:(hf + 1) * 256], stat, mov,
                        start=(cp == 0 and hf % 2 == 0), stop=(cp == 1),
                        perf_mode=DR, skip_group_check=True)
            sl = slice(piece * 1024, (piece + 1) * 1024)
            if qk == 0:
                nc.vector.tensor_scalar(qsb[g][:, sl], p, 1.0 / SW, bqs[g],
                                        ALU.mult, ALU.add)
            else:
                nc.vector.tensor_scalar_mul(ksb[g][:, sl], p, 1.0 / SW)

        def v_piece(t):
            # m-chunk pair (2t, 2t+1) -> vsb[t]: all 8 heads' V, e4m3
            p = ps_s.tile([128, 1024], F32, tag="ps_s", name=f"v{t}")
            for i in range(2):
                for cp in range(2):
                    stat = _ap3(xft, (2 * cp) * N + (2 * t + i) * 128,
                                N, 128)
                    for dw in range(2):
                        nc.tensor.matmul(
                            p[:, i * 512 + dw * 256:i * 512 + dw * 256 + 256],
                            stat, _ap3(wvt, cp * 1024 + dw * 256, 512, 256),
                            start=(cp == 0 and dw == 0), stop=(cp == 1),
                            perf_mode=DR, skip_group_check=True)
            nc.scalar.activation(vsb[t], p, AF.Copy, scale=1.0 / SW)

        def s_chunk(h, mt, pe):
            # S^T chunk [128 m, 1024 n] for head h + exp into pair tile half
            g, lo = h // 2, (h % 2) * 64
            sp = ps_s.tile([128, 1024], F32, tag="ps_s",
                           name=f"s{h}_{mt}")
            for ntt in range(2):
                nc.tensor.matmul(
                    sp[:, ntt * 512:(ntt + 1) * 512],
                    ksb[g][lo:lo + 64, mt * 128:(mt + 1) * 128],
                    qsb[g][lo:lo + 64, ntt * 512:(ntt + 1) * 512],
                    start=True, stop=True, tile_position=(lo, 0))
            half = (mt % 2) * 1024
            idx = h * MT + mt
            is_act = ((idx + 1) * ACT_NUM) // ACT_DEN \
                != (idx * ACT_NUM) // ACT_DEN
            if is_act:
                nc.scalar.activation(pe[:, half:half + 1024], sp, AF.Exp,
                                     scale=0.125, bias=ebias[:])
            else:
                nc.vector.tensor_scalar(
                    pe[:, half:half + 1024].bitcast(I8), sp,
                    EXP_A, EXP_B, ALU.mult, ALU.add)

        def av_pair(h, t, pe, avp, zp):
            for nb in range(NB):
                stat = _ap3(pe, nb * 128, 1024, 128)
                nc.tensor.matmul(
                    avp[:, nb * 64:(nb + 1) * 64], stat,
                    _ap3(vsb[t], h * 64, 512, 64),
                    start=(t == 0 and nb == 0), stop=(t == MT // 2 - 1),
                    perf_mode=DR, skip_group_check=True)
                nc.tensor.matmul(
                    zp[:, nb:nb + 1], stat,
                    bass.AP(tensor=ones2[:].tensor, offset=ones2[:].offset,
                            ap=[[ones2[:].ap[0][0], 128], [1, 2], [1, 1]]),
                    start=(t == 0 and nb == 0), stop=(t == MT // 2 - 1),
                    perf_mode=DR, skip_group_check=True)

        def norm(h, avp, zp, asb):
            lo = (h % 2) * 64
            rz = rzp.tile([128, NB], F32, tag="rz", name=f"rz{h}")
            nc.vector.reciprocal(rz, zp)
            for nb in range(NB):
                o = nb * 128 + lo
                if h % 2 == 0:
                    nc.scalar.activation(asb[:, o:o + 64],
                                         avp[:, nb * 64:(nb + 1) * 64],
                                         AF.Copy, scale=rz[:, nb:nb + 1])
                else:
                    nc.vector.tensor_scalar_mul(
                        asb[:, o:o + 64], avp[:, nb * 64:(nb + 1) * 64],
                        rz[:, nb:nb + 1])

        def group_end(g, asb):
            tp = ps_av.tile([128, NSL], BF16, tag="ps_av", name=f"tp{g}")
            for nb in range(NB):
                nc.tensor.matmul(
                    tp[:, nb * 128:(nb + 1) * 128],
                    asb[:, nb * 128:(nb + 1) * 128], ident,
                    is_transpose=True, start=(nb == 0), stop=(nb == NB - 1),
                    skip_group_check=True)
            half = (g % 2) * 1024
            nc.scalar.activation(aoT[g // 2][:, half:half + 1024], tp,
                                 AF.Identity, bias=bvs[g], scale=1.0)

        def out_proj(ot):
            po = ps_s.tile([128, 1024], F32, tag="ps_s", name=f"op{ot}")
            for cp in range(2):
                stat = _ap3(wot, (cp * 4 + ot) * 256, 128, 128)
                for hf in range(4):
                    mov = _ap3(aoT[cp], hf * 256, 1024, 256)
                    nc.tensor.matmul(
                        po[:, hf * 256:(hf + 1) * 256], stat, mov,
                        start=(cp == 0 and hf % 2 == 0), stop=(cp == 1),
                        perf_mode=DR, skip_group_check=True)
            t2 = epi.tile([128, NSL], F32, tag="t2")
            nc.vector.scalar_tensor_tensor(
                out=t2, in0=po, scalar=gam128, in1=xrs[ot],
                op0=ALU.mult, op1=ALU.add)
            nc.sync.dma_start(out=out[ot * 128:(ot + 1) * 128, :], in_=t2)

        # ---- preamble ----------------------------------------------------
        kq_piece(0, 0, 0)            # Q(0)
        kq_piece(0, 0, 1)            # K(0) piece 0 (m 0..1023)
        v_piece(0)
        v_piece(1)

        # ---- attention ---------------------------------------------------
        asb = None
        for h in range(NH):
            g = h // 2
            avp = ps_av.tile([128, NB * 64], F32, tag="ps_av",
                             name=f"av{h}")
            zp = ps_z.tile([128, NB], F32, tag="ps_z", name=f"z{h}")
            if h % 2 == 0:
                asb = asbp.tile([128, NSL], BF16, tag="asb",
                                name=f"asb{g}")
            pes = {}
            pe = None
            for mt in range(MT):
                if mt % 2 == 0:
                    pe = pexp.tile([128, 2048], E5, tag="pe",
                                   name=f"pe{h}_{mt}")
                    pes[mt // 2] = pe
                s_chunk(h, mt, pe)
                # interleaved projections
                if h == 0:
                    if mt % 8 == 4 and mt // 8 < 3:
                        kq_piece(0, mt // 8 + 1, 1)  # K(0) pieces 1..3
                    if mt % 2 == 0 and 2 + mt // 2 < MT // 2:
                        v_piece(2 + mt // 2)         # V pairs 2..15
                if h % 2 == 1 and h < NH - 1:
                    gn = g + 1
                    if mt == 0:
                        kq_piece(gn, 0, 0)           # Q(g+1)
                    elif mt % 4 == 2 and mt < 16:
                        kq_piece(gn, mt // 4, 1)     # K(g+1) pieces 0..3
                # AV two pairs behind the S fill
                if mt >= 5 and mt % 2 == 1:
                    av_pair(h, (mt - 5) // 2, pes[(mt - 5) // 2], avp, zp)
            for t in range(MT // 2 - 2, MT // 2):
                av_pair(h, t, pes[t], avp, zp)
            norm(h, avp, zp, asb)
            if h % 2 == 1:
                group_end(g, asb)

        # ---- output projection ------------------------------------------
        for ot in range(4):
            out_proj(ot)


def _prep_in_maps(x, w_qkv, b_qkv, w_out, b_out, gamma):
    x = np.asarray(x, np.float32).reshape(B, C, N)
    w_qkv = np.asarray(w_qkv, np.float32)
    b_qkv = np.asarray(b_qkv, np.float32)
    w_out = np.asarray(w_out, np.float32)
    b_out = np.asarray(b_out, np.float32)
    gamma = np.asarray(gamma, np.float32)

    # wqkf[p, ((g*2+qk)*2+cp)*256 + i*128 + d] = w_qkv[qk*C+g*128+d, c]*SW
    # with c = p + 128*i + 256*cp
    t = (w_qkv[:2 * C] * SW).reshape(2, 4, 128, 2, 2, 128)  # qk g d cp i p
    wqkf = np.ascontiguousarray(
        t.transpose(5, 1, 0, 3, 4, 2).reshape(128, 4096)).astype(E4n)
    # wvf[p, cp*1024 + i*512 + dall] = w_qkv[2C+dall, c]*SW
    t = (w_qkv[2 * C:] * SW).reshape(512, 2, 2, 128)        # dall cp i p
    wvf = np.ascontiguousarray(
        t.transpose(3, 1, 2, 0).reshape(128, 2048)).astype(E4n)
    # wof[p, (cp*4+ot)*256 + i*128 + o] = w_out[ot*128+o, c]*SW
    t = (w_out * SW).reshape(4, 128, 2, 2, 128)             # ot o cp i p
    wof = np.ascontiguousarray(
        t.transpose(4, 2, 0, 3, 1).reshape(128, 2048)).astype(E4n)

    bqcol = b_qkv[:C].reshape(C, 1)
    bvcol = b_qkv[2 * C:].reshape(C, 1)
    gam = (gamma / SW).reshape(1, 1)

    in_maps = []
    for i in range(8):
        b, q = i // 4, i % 4
        rot = np.roll(x[b], -q * NSL, axis=1)
        xf8 = np.ascontiguousarray(
            rot.reshape(4, 128, N).transpose(1, 0, 2).reshape(128, 4 * N)
        ).astype(E4n)
        xrs = np.ascontiguousarray(x[b][:, q * NSL:(q + 1) * NSL]) \
            + (gamma.reshape(()) * b_out)[:, None].astype(np.float32)
        in_maps.append({
            "xf": xf8, "xr": xrs, "wqkf": wqkf, "wvf": wvf, "wof": wof,
            "bq": bqcol, "bvc": bvcol, "gam": gam,
        })
    return in_maps


def _assemble(results):
    full = np.empty((B, C, N), np.float32)
    for i in range(8):
        b, q = i // 4, i % 4
        full[b][:, q * NSL:(q + 1) * NSL] = results[i]["out"]
    return full.reshape(B, C, H, W)


def kernel(x, w_qkv, b_qkv, w_out, b_out, gamma):
    if "nc" not in _cached:
        _cached["nc"] = _build_kernel()
    nc = _cached["nc"]
    in_maps = _prep_in_maps(x, w_qkv, b_qkv, w_out, b_out, gamma)
    res = run_bass_kernel_spmd(nc, in_maps, core_ids=list(range(8)))
    return _assemble(res.results)


# revision 16
# speedup vs baseline: 1.2699x; 1.0864x over previous
"""Multi-head self-attention (B=2, C=512, H=W=64, 8 heads) on 8 TRN2 cores.

Sharding: core i handles batch b = i//4 and pixel quarter q = i%4 (1024 of
4096 pixels) for ALL 8 heads.  K/V projections cover the full pixel range
(attention context), Q only the local quarter.  No collectives.

fp8 DoubleRow design (vs the bf16 baseline): all projections and the AV
matmul run in fp8 with perf_mode=DoubleRow (0.5 cycles/row, 2x contraction
per instruction -> 4x cheaper than bf16 on contraction-split matmuls).

  x      host-cast to e4m3 and c-folded: xf[p, cb*4096+n] = x[cb*128+p, n]
         so a DR matmul contracts c-pairs (c = p + 128i + 256cp) via 3-D
         APs [[pitch,128],[4096,2],[1,n]].  Weights are host-scaled x16
         (lifts them out of e4m3's subnormal range) and host-folded.
  K,Q:   DR matmuls, out [d(128), m]; evac: tensor_scalar x(1/16) (+bq for
         Q) -> bf16.  K bias dropped (softmax-invariant).
  V:     DR, out [m(128), d-all-heads]; evac ACT Copy x(1/16) -> e4m3 into
         pair tiles vsb[t][p, i*512 + h*64 + d] (i = chunk parity).
  S^T:   bf16 as in the baseline (2 heads row-tiled per group); per-head
         processing, chunks of 128 keys -> PSUM [128, 1024] f32.
  exp:   P = exp(s/8 - ln8) -> e5m2 (shift keeps P <= 137 inside e5m2/e4m3
         range; any fixed shift cancels in softmax).  Split ACT (true Exp,
         scale+bias, e5m2 out) / DVE (Schraudolph: bits = round(A*s + B)
         as int8, bitcast e5m2; HW-verified bit-exact vs numpy).  Both
         write halves of a pair tile pexp[p, i*1024 + n].
  AV:    DR e5m2 x e4m3: stationary P [[pitch,128],[1024,2],[1,128]],
         moving V [[pitch,128],[512,2],[1,64]], accumulating [n,d] over 16
         m-pair chunks; Z via DR ones-moving [[pitch,128],[1,2],[1,1]].
  norm:  rz = reciprocal(Z); per-nb scale into asb (bf16); v-bias deferred
         to the transpose evac (per-partition there).
  out:   PE-transpose asb -> [c, n] bf16 PSUM; evac +bv -> aoT e4m3 with
         c-pairs in column halves (aoT[cp][p, i*1024+n]); out-proj DR;
         epilogue (psum * gamma/16 + xr) -> DMA out.

PSUM (8 banks): S 2x[128,1024]f32 = 4, AV 2x1 (reused for the transpose),
Z 1, proj transients 1.
"""

import numpy as np
import ml_dtypes

import concourse.bass as bass
import concourse.mybir as mybir
import concourse.tile as tile
from concourse import bacc
from concourse import masks
from concourse.bass_utils import run_bass_kernel_spmd

F32 = mybir.dt.float32
BF16 = mybir.dt.bfloat16
E4 = mybir.dt.float8e4
E5 = mybir.dt.float8e5
I8 = mybir.dt.int8
AF = mybir.ActivationFunctionType
ALU = mybir.AluOpType
DR = mybir.MatmulPerfMode.DoubleRow

E4n = ml_dtypes.float8_e4m3
E5n = ml_dtypes.float8_e5m2

B, C, H, W = 2, 512, 64, 64
N = H * W              # 4096 pixels
NH, HD = 8, 64         # heads, head dim
NSL = N // 4           # pixels per core (queries)
NG = NH // 2           # head groups of 2
MT = N // 128          # m-chunks (key pixels)
NB = NSL // 128        # n-blocks (AV output partitions)
SW = 16.0              # host weight scale (e4m3 subnormal dodge)
SHIFT = float(np.log(8.0))          # P = exp(s/8 - ln8): e5m2-safe
EXP_A = 4.0 * float(np.log2(np.e)) / 8.0   # Schraudolph (1/8 folded)
EXP_B = 60.0 - 12.0 + 0.44                  # e5m2 bias - 4*shift + rounding

# exp engine split: ACT gets ACT_NUM of every ACT_DEN chunks (rest DVE)
ACT_NUM, ACT_DEN = 140, 256

_cached = {}


def _ap3(t, offset, step2, inner):
    """3-D folded view [[pitch,128],[step2,2],[1,inner]] of tile t."""
    a = t[:]
    return bass.AP(tensor=a.tensor, offset=a.offset + offset,
                   ap=[[a.ap[0][0], 128], [step2, 2], [1, inner]])


def _build_kernel():
    nc = bacc.Bacc("TRN2", target_bir_lowering=False, debug=False,
                   num_devices=8)

    xf = nc.dram_tensor("xf", [128, 4 * N], E4, kind="ExternalInput")
    xr = nc.dram_tensor("xr", [C, NSL], F32, kind="ExternalInput")
    wqkf = nc.dram_tensor("wqkf", [128, 4096], E4, kind="ExternalInput")
    wvf = nc.dram_tensor("wvf", [128, 2048], E4, kind="ExternalInput")
    wof = nc.dram_tensor("wof", [128, 2048], E4, kind="ExternalInput")
    bqv = nc.dram_tensor("bqv", [C, 3], F32, kind="ExternalInput")
    out = nc.dram_tensor("out", [C, NSL], F32, kind="ExternalOutput")

    with tile.TileContext(nc) as tc:
        _emit_body(nc, tc, xf, xr, wqkf, wvf, wof, bqv, out)
    nc.compile()
    return nc


def _emit_body(nc, tc, xf, xr, wqkf, wvf, wof, bqv, out):
    from contextlib import ExitStack
    with ExitStack() as ctx:
        ep = ctx.enter_context

        consts = ep(tc.tile_pool(name="consts", bufs=1))
        xrp = ep(tc.tile_pool(name="xrp", bufs=1))
        kq = ep(tc.tile_pool(name="kq", bufs=1))
        vp = ep(tc.tile_pool(name="vp", bufs=1))
        pexp = ep(tc.tile_pool(name="pexp", bufs=8))
        asbp = ep(tc.tile_pool(name="asbp", bufs=2))
        rzp = ep(tc.tile_pool(name="rzp", bufs=4))
        aop = ep(tc.tile_pool(name="aop", bufs=1))
        epi = ep(tc.tile_pool(name="epi", bufs=4))
        # ONE shared 3-slot rotation ([128,1024] f32 = 2 banks each) for S
        # chunks AND all projection transients: breaks the 2-slot WAR chain
        # (fill(mt+k) waits exp(mt)) that serialized fill+exp latencies.
        ps_s = ep(tc.tile_pool(name="ps_s", bufs=3, space="PSUM"))
        ps_av = ep(tc.tile_pool(name="ps_av", bufs=1, space="PSUM"))
        ps_z = ep(tc.tile_pool(name="ps_z", bufs=1, space="PSUM"))

        # ---- constants + input loads (DMAs batched: HWDGE desc-gen costs
        # ~632ns per DMA on one exclusive device) ---------------------------
        xft = consts.tile([128, 4 * N], E4, tag="xft")
        for cb in range(4):   # queries + K piece 0 (m 0..1023) first
            o = cb * N
            nc.sync.dma_start(out=xft[:, o:o + 1024], in_=xf[:, o:o + 1024])
        wqkt = consts.tile([128, 4096], E4, tag="wqkt")
        nc.sync.dma_start(out=wqkt, in_=wqkf[:, :])
        bqvs = []
        for g in range(NG):
            t = consts.tile([128, 3], F32, tag=f"bqv{g}", name=f"bqvt{g}")
            nc.gpsimd.dma_start(out=t, in_=bqv[g * 128:(g + 1) * 128, :])
            bqvs.append(t)
        bqs = [t[:, 0:1] for t in bqvs]
        bvs = [t[:, 1:2] for t in bqvs]
        gam128 = bqvs[0][:, 2:3]
        for cb in range(4):   # key tail
            o = cb * N + 1024
            nc.sync.dma_start(out=xft[:, o:o + 3072],
                              in_=xf[:, o:o + 3072])
        wvt = consts.tile([128, 2048], E4, tag="wvt")
        nc.sync.dma_start(out=wvt, in_=wvf[:, :])
        wot = consts.tile([128, 2048], E4, tag="wot")
        nc.sync.dma_start(out=wot, in_=wof[:, :])
        ebias = consts.tile([128, 1], F32, tag="ebias")
        nc.vector.memset(ebias, -SHIFT)
        ones2 = consts.tile([128, 2], E4, tag="ones2")
        nc.vector.memset(ones2, 1.0)
        ident = consts.tile([128, 128], BF16, tag="ident")
        masks.make_identity(nc, ident[:])
        xrs = []
        for cb in range(4):
            t = xrp.tile([128, NSL], F32, tag=f"xr{cb}", name=f"xrt{cb}")
            nc.sync.dma_start(out=t, in_=xr[cb * 128:(cb + 1) * 128, :])
            xrs.append(t)

        # ---- persistent SBUF state --------------------------------------
        ksb = [kq.tile([128, N], BF16, tag=f"k{g}", name=f"k{g}")
               for g in range(NG)]
        qsb = [kq.tile([128, NSL], BF16, tag=f"q{g}", name=f"q{g}")
               for g in range(NG)]
        vsb = [vp.tile([128, 1024], E4, tag=f"v{t}", name=f"v{t}")
               for t in range(MT // 2)]
        aoT = [aop.tile([128, 2048], E4, tag=f"aoT{cp}", name=f"aoT{cp}")
               for cp in range(2)]

        def kq_piece(g, piece, qk):
            # Q (qk=0, one 1024-n piece) / K (qk=1, pieces 0..3 of 1024 m)
            p = ps_s.tile([128, 1024], F32, tag="ps_s",
                          name=f"kq{qk}_{g}_{piece}")
            for cp in range(2):
                stat = _ap3(wqkt, ((g * 2 + qk) * 2 + cp) * 256, 128, 128)
                for hf in range(4):
                    mov = _ap3(xft, (2 * cp) * N + piece * 1024 + hf * 256,
                               N, 256)
                    nc.tensor.matmul(
                        p[:, hf * 256:(hf + 1) * 256], stat, mov,
                        start=(cp == 0 and hf % 2 == 0), stop=(cp == 1),
                        perf_mode=DR, skip_group_check=True)
            sl = slice(piece * 1024, (piece + 1) * 1024)
            if qk == 0:
                nc.vector.tensor_scalar(qsb[g][:, sl], p, 1.0 / SW, bqs[g],
                                        ALU.mult, ALU.add)
            else:
                nc.vector.tensor_scalar_mul(ksb[g][:, sl], p, 1.0 / SW)

        def v_piece(t):
            # m-chunk pair (2t, 2t+1) -> vsb[t]: all 8 heads' V, e4m3
            p = ps_s.tile([128, 1024], F32, tag="ps_s", name=f"v{t}")
            for i in range(2):
                for cp in range(2):
                    stat = _ap3(xft, (2 * cp) * N + (2 * t + i) * 128,
                                N, 128)
                    for dw in range(2):
                        nc.tensor.matmul(
                            p[:, i * 512 + dw * 256:i * 512 + dw * 256 + 256],
                            stat, _ap3(wvt, cp * 1024 + dw * 256, 512, 256),
                            start=(cp == 0 and dw == 0), stop=(cp == 1),
                            perf_mode=DR, skip_group_check=True)
            nc.scalar.activation(vsb[t], p, AF.Copy, scale=1.0 / SW)

        def s_chunk(h, mt, pe):
            # S^T chunk [128 m, 1024 n] for head h + exp into pair tile half
            g, lo = h // 2, (h % 2) * 64
            sp = ps_s.tile([128, 1024], F32, tag="ps_s",
                           name=f"s{h}_{mt}")
            for ntt in range(2):
                nc.tensor.matmul(
                    sp[:, ntt * 512:(ntt + 1) * 512],
                    ksb[g][lo:lo + 64, mt * 128:(mt + 1) * 128],
                    qsb[g][lo:lo + 64, ntt * 512:(ntt + 1) * 512],
                    start=True, stop=True, tile_position=(lo, 0))
            half = (mt % 2) * 1024
            idx = h * MT + mt
            is_act = ((idx + 1) * ACT_NUM) // ACT_DEN \
                != (idx * ACT_NUM) // ACT_DEN
            if is_act:
                nc.scalar.activation(pe[:, half:half + 1024], sp, AF.Exp,
                                     scale=0.125, bias=ebias[:])
            else:
                nc.vector.tensor_scalar(
                    pe[:, half:half + 1024].bitcast(I8), sp,
                    EXP_A, EXP_B, ALU.mult, ALU.add)

        def av_pair(h, t, pe, avp, zp):
            for nb in range(NB):
                stat = _ap3(pe, nb * 128, 1024, 128)
                nc.tensor.matmul(
                    avp[:, nb * 64:(nb + 1) * 64], stat,
                    _ap3(vsb[t], h * 64, 512, 64),
                    start=(t == 0 and nb == 0), stop=(t == MT // 2 - 1),
                    perf_mode=DR, skip_group_check=True)
                nc.tensor.matmul(
                    zp[:, nb:nb + 1], stat,
                    bass.AP(tensor=ones2[:].tensor, offset=ones2[:].offset,
                            ap=[[ones2[:].ap[0][0], 128], [1, 2], [1, 1]]),
                    start=(t == 0 and nb == 0), stop=(t == MT // 2 - 1),
                    perf_mode=DR, skip_group_check=True)

        def norm(h, avp, zp, asb):
            lo = (h % 2) * 64
            rz = rzp.tile([128, NB], F32, tag="rz", name=f"rz{h}")
            nc.vector.reciprocal(rz, zp)
            for nb in range(NB):
                o = nb * 128 + lo
                if h % 2 == 0:
                    nc.scalar.activation(asb[:, o:o + 64],
                                         avp[:, nb * 64:(nb + 1) * 64],
                                         AF.Copy, scale=rz[:, nb:nb + 1])
                else:
                    nc.vector.tensor_scalar_mul(
                        asb[:, o:o + 64], avp[:, nb * 64:(nb + 1) * 64],
                        rz[:, nb:nb + 1])

        def group_end(g, asb):
            tp = ps_av.tile([128, NSL], BF16, tag="ps_av", name=f"tp{g}")
            for nb in range(NB):
                nc.tensor.matmul(
                    tp[:, nb * 128:(nb + 1) * 128],
                    asb[:, nb * 128:(nb + 1) * 128], ident,
                    is_transpose=True, start=(nb == 0), stop=(nb == NB - 1),
                    skip_group_check=True)
            half = (g % 2) * 1024
            nc.scalar.activation(aoT[g // 2][:, half:half + 1024], tp,
                                 AF.Identity, bias=bvs[g], scale=1.0)

        def out_proj(ot):
            po = ps_s.tile([128, 1024], F32, tag="ps_s", name=f"op{ot}")
            for cp in range(2):
                stat = _ap3(wot, (cp * 4 + ot) * 256, 128, 128)
                for hf in range(4):
                    mov = _ap3(aoT[cp], hf * 256, 1024, 256)
                    nc.tensor.matmul(
                        po[:, hf * 256:(hf + 1) * 256], stat, mov,
                        start=(cp == 0 and hf % 2 == 0), stop=(cp == 1),
                        perf_mode=DR, skip_group_check=True)
            t2 = epi.tile([128, NSL], F32, tag="t2")
            nc.vector.scalar_tensor_tensor(
                out=t2, in0=po, scalar=gam128, in1=xrs[ot],
                op0=ALU.mult, op1=ALU.add)
            nc.sync.dma_start(out=out[ot * 128:(ot + 1) * 128, :], in_=t2)

        # ---- preamble ----------------------------------------------------
        kq_piece(0, 0, 0)            # Q(0)
        kq_piece(0, 0, 1)            # K(0) piece 0 (m 0..1023)
        v_piece(0)
        v_piece(1)

        # ---- attention ---------------------------------------------------
        asb = None
        for h in range(NH):
            g = h // 2
            avp = ps_av.tile([128, NB * 64], F32, tag="ps_av",
                             name=f"av{h}")
            zp = ps_z.tile([128, NB], F32, tag="ps_z", name=f"z{h}")
            if h % 2 == 0:
                asb = asbp.tile([128, NSL], BF16, tag="asb",
                                name=f"asb{g}")
            pes = {}
            pe = None
            for mt in range(MT):
                if mt % 2 == 0:
                    pe = pexp.tile([128, 2048], E5, tag="pe",
                                   name=f"pe{h}_{mt}")
                    pes[mt // 2] = pe
                s_chunk(h, mt, pe)
                # interleaved projections
                if h == 0:
                    if mt % 8 == 4 and mt // 8 < 3:
                        kq_piece(0, mt // 8 + 1, 1)  # K(0) pieces 1..3
                    if mt % 2 == 0 and 2 + mt // 2 < MT // 2:
                        v_piece(2 + mt // 2)         # V pairs 2..15
                if h % 2 == 1 and h < NH - 1:
                    gn = g + 1
                    if mt == 0:
                        kq_piece(gn, 0, 0)           # Q(g+1)
                    elif mt % 4 == 2 and mt < 16:
                        kq_piece(gn, mt // 4, 1)     # K(g+1) pieces 0..3
                # AV three pairs behind the S fill
                if mt >= 7 and mt % 2 == 1:
                    av_pair(h, (mt - 7) // 2, pes[(mt - 7) // 2], avp, zp)
            for t in range(MT // 2 - 3, MT // 2):
                av_pair(h, t, pes[t], avp, zp)
            norm(h, avp, zp, asb)
            if h % 2 == 1:
                group_end(g, asb)

        # ---- output projection ------------------------------------------
        for ot in range(4):
            out_proj(ot)


def _prep_in_maps(x, w_qkv, b_qkv, w_out, b_out, gamma):
    x = np.asarray(x, np.float32).reshape(B, C, N)
    w_qkv = np.asarray(w_qkv, np.float32)
    b_qkv = np.asarray(b_qkv, np.float32)
    w_out = np.asarray(w_out, np.float32)
    b_out = np.asarray(b_out, np.float32)
    gamma = np.asarray(gamma, np.float32)

    # wqkf[p, ((g*2+qk)*2+cp)*256 + i*128 + d] = w_qkv[qk*C+g*128+d, c]*SW
    # with c = p + 128*i + 256*cp
    t = (w_qkv[:2 * C] * SW).reshape(2, 4, 128, 2, 2, 128)  # qk g d cp i p
    wqkf = np.ascontiguousarray(
        t.transpose(5, 1, 0, 3, 4, 2).reshape(128, 4096)).astype(E4n)
    # wvf[p, cp*1024 + i*512 + dall] = w_qkv[2C+dall, c]*SW
    t = (w_qkv[2 * C:] * SW).reshape(512, 2, 2, 128)        # dall cp i p
    wvf = np.ascontiguousarray(
        t.transpose(3, 1, 2, 0).reshape(128, 2048)).astype(E4n)
    # wof[p, (cp*4+ot)*256 + i*128 + o] = w_out[ot*128+o, c]*SW
    t = (w_out * SW).reshape(4, 128, 2, 2, 128)             # ot o cp i p
    wof = np.ascontiguousarray(
        t.transpose(4, 2, 0, 3, 1).reshape(128, 2048)).astype(E4n)

    bqv = np.stack([b_qkv[:C], b_qkv[2 * C:],
                    np.full(C, float(gamma.reshape(())) / SW,
                            np.float32)], axis=1).astype(np.float32)

    in_maps = []
    for i in range(8):
        b, q = i // 4, i % 4
        rot = np.roll(x[b], -q * NSL, axis=1)
        xf8 = np.ascontiguousarray(
            rot.reshape(4, 128, N).transpose(1, 0, 2).reshape(128, 4 * N)
        ).astype(E4n)
        xrs = np.ascontiguousarray(x[b][:, q * NSL:(q + 1) * NSL]) \
            + (gamma.reshape(()) * b_out)[:, None].astype(np.float32)
        in_maps.append({
            "xf": xf8, "xr": xrs, "wqkf": wqkf, "wvf": wvf, "wof": wof,
            "bqv": bqv,
        })
    return in_maps


def _assemble(results):
    full = np.empty((B, C, N), np.float32)
    for i in range(8):
        b, q = i // 4, i % 4
        full[b][:, q * NSL:(q + 1) * NSL] = results[i]["out"]
    return full.reshape(B, C, H, W)


def kernel(x, w_qkv, b_qkv, w_out, b_out, gamma):
    if "nc" not in _cached:
        _cached["nc"] = _build_kernel()
    nc = _cached["nc"]
    in_maps = _prep_in_maps(x, w_qkv, b_qkv, w_out, b_out, gamma)
    res = run_bass_kernel_spmd(nc, in_maps, core_ids=list(range(8)))
    return _assemble(res.results)
